# revision 1
# baseline (speedup 1.0000x reference)
"""GAT layer (dense adjacency) on 8 Trainium2 NeuronCores.

Problem: H = elu(softmax_j(mask(A, leaky_relu(Wh1_i + Wh2_j))) @ Wh),
A: [8, 2048, 2048] 0/1 f32, X: [8, 2048, 64], Ws: [64, 64], a: [128, 1].

Sharding: data-parallel over batch B=8 -> one batch element per core.

Per-core device algorithm (transposed layout, rows = source node j,
cols = destination node i):
  - Host precomputes Wh = X@Ws, Wh1 = Wh@a1, Wh2 = Wh@a2 (tiny: 0.1% of work).
  - For each j-tile (128 rows) the masked logits are built in PSUM by the
    tensor engine itself:
        P[j, i] = ones2^T @ [Wh1_hi; Wh1_lo]  (broadcast of Wh1 along j)
                + (A_block)^T @ (C*I)         (mask: C=512 where edge, 0 else)
    so no elementwise mask multiply and no separate transpose of the
    attention matrix is ever needed (the matmul with C*I transposes A).
  - ACT: e = LeakyRelu(P + (Wh2[j] - C)), pa = Exp(e - S) -> fp16.
    Where A=0 the logit is z - 512 -> exp ~ 0.  Where A=1 it is exactly z.
    S is a host-computed shift keeping pa in fp16 range; softmax scale
    invariance cancels it.
  - H^T[d, i] (+ row sums s_i via an appended ones column) accumulate on the
    tensor engine in fp16: H^T = sum_t WhAug_t^T @ pa_t.
  - Epilogue: PE-transpose H^T 128-col chunks back to [128, 65], 1/s via DVE
    reciprocal, H = elu(H_pre * (1/s)) built from Exp/min/relu ops.
Optionally a subset of tiles computes the LeakyRelu on the vector engine
(3 tensor_scalar/tensor_tensor ops) to balance ACT vs DVE.
"""
import sys

for _p in ("/opt/trn_rl_repo",):
    if _p not in sys.path:
        sys.path.append(_p)

import numpy as np
import ml_dtypes

import concourse.bass as bass
import concourse.bacc as bacc
import concourse.tile as tile
from concourse import mybir
from concourse import bass_utils

F32 = mybir.dt.float32
BF16 = mybir.dt.bfloat16
FP16 = mybir.dt.float16
AF = mybir.ActivationFunctionType
ALU = mybir.AluOpType

B, N, F, D = 8, 2048, 64, 64
NT = N // 128          # 16 j-tiles / i-tiles
HALF = N // 2          # 1024 columns processed per psum tile
C_MASK = 512.0
ALPHA = 0.2
# (h, t) pairs whose LeakyRelu runs on DVE instead of ACT (engine balance).
DVE_LR = {(h, t) for h in range(2) for t in range(NT) if t % 2 == 0}

_CACHED = {}


def _build_program():
    nc = bacc.Bacc("TRN2", target_bir_lowering=False, debug=False)

    A_d = nc.dram_tensor("A", [N, N], F32, kind="ExternalInput")
    whaug_d = nc.dram_tensor("WhAug16", [N, D + 1], FP16, kind="ExternalInput")
    wh1p_d = nc.dram_tensor("wh1p", [2, N], BF16, kind="ExternalInput")
    biasT_d = nc.dram_tensor("biasT", [128, NT], F32, kind="ExternalInput")
    negS_d = nc.dram_tensor("negS", [128, 1], F32, kind="ExternalInput")
    ci_d = nc.dram_tensor("CI", [128, 128], BF16, kind="ExternalInput")
    ones2_d = nc.dram_tensor("ones2", [2, 128], BF16, kind="ExternalInput")
    i65_d = nc.dram_tensor("I65", [D + 1, D + 1], F32, kind="ExternalInput")
    H_d = nc.dram_tensor("H", [N, D], F32, kind="ExternalOutput")

    with tile.TileContext(nc) as tc:
        with tc.tile_pool(name="const", bufs=1) as cp, \
             tc.tile_pool(name="aslab", bufs=NT) as ap_pool, \
             tc.tile_pool(name="work", bufs=3) as wp, \
             tc.tile_pool(name="outp", bufs=1) as op_pool, \
             tc.tile_pool(name="psP", bufs=2, space="PSUM") as psP, \
             tc.tile_pool(name="psH", bufs=1, space="PSUM") as psH, \
             tc.tile_pool(name="psT", bufs=2, space="PSUM") as psT:

            # ---- constants / small inputs ----
            whaug = cp.tile([128, NT * (D + 1)], FP16, name="whaug")
            nc.sync.dma_start(
                whaug[:].rearrange("p (t c) -> p t c", c=D + 1),
                whaug_d.ap().rearrange("(t p) c -> p t c", p=128),
            )
            wh1p = cp.tile([2, N], BF16, name="wh1p")
            nc.sync.dma_start(wh1p[:], wh1p_d.ap())
            biasT = cp.tile([128, NT], F32, name="biasT")
            nc.sync.dma_start(biasT[:], biasT_d.ap())
            negS = cp.tile([128, 1], F32, name="negS")
            nc.sync.dma_start(negS[:], negS_d.ap())
            ci = cp.tile([128, 128], BF16, name="ci")
            nc.sync.dma_start(ci[:], ci_d.ap())
            ones2 = cp.tile([2, 128], BF16, name="ones2")
            nc.sync.dma_start(ones2[:], ones2_d.ap())
            i65 = cp.tile([D + 1, D + 1], F32, name="i65")
            nc.sync.dma_start(i65[:], i65_d.ap())
            alpha02 = cp.tile([128, 1], F32, name="alpha02")
            nc.vector.memset(alpha02[:], ALPHA)
            # preload ACT table set (Exp/Prelu share one set) during input DMA
            warm = cp.tile([1, 1], F32, name="warm")
            nc.vector.memset(warm[:], 0.0)
            warm2 = cp.tile([1, 1], F32, name="warm2")
            nc.scalar.activation(warm2[:], warm[:], AF.Exp, bias=0.0, scale=1.0)

            # ---- A column-slabs, f32 via HWDGE; the mask matmuls read a
            # stride-2 bf16 view (A is exactly 0/1, so the f32 high halves
            # ARE the bf16 values - no cast DMA needed) ----
            # aslab[t][p, 128*r + q] = A[128*r + p, 128*t + q]
            aslabs = []
            for t in range(NT):
                sl = ap_pool.tile([128, N], F32, name=f"aslab{t}", tag="aslab")
                nc.sync.dma_start(
                    sl[:].rearrange("p (r q) -> p r q", q=128),
                    A_d.ap()[:, 128 * t:128 * (t + 1)]
                    .rearrange("(r p) q -> p r q", p=128),
                )
                aslabs.append(sl)

            hout = op_pool.tile([128, NT * D], F32, name="hout")

            for h in range(2):
                ht_ps = psH.tile([D + 1, HALF], F32, name="ht_ps", tag="ht_ps")

                def fill(t, h=h):
                    pp = psP.tile([128, HALF], F32, name="pp", tag="pp")
                    # broadcast of Wh1 (hi+lo rows, K=2) along partitions
                    for c in range(2):
                        nc.tensor.matmul(
                            pp[:, 512 * c:512 * (c + 1)],
                            ones2[:],
                            wh1p[:, HALF * h + 512 * c:HALF * h + 512 * (c + 1)],
                            start=True, stop=False,
                        )
                    # mask + transpose: pp[:, 128k:+128] += C * A_block^T
                    for k in range(8):
                        r = 8 * h + k
                        nc.tensor.matmul(
                            pp[:, 128 * k:128 * (k + 1)],
                            aslabs[t][:].bitcast(BF16)[:, 256 * r + 1:256 * (r + 1):2],
                            ci[:],
                            start=False, stop=True, skip_group_check=True,
                        )
                    return pp

                pp_next = fill(0)
                for t in range(NT):
                    pp = pp_next
                    if t + 1 < NT:
                        pp_next = fill(t + 1)
                    pa = wp.tile([128, HALF], FP16, name="pa", tag="pa")
                    if (h, t) in DVE_LR:
                        # LeakyRelu on DVE: z = pp + bias; e = max(z, 0.2 z)
                        z1 = wp.tile([128, HALF], F32, name="z1", tag="z1")
                        nc.vector.tensor_scalar(
                            z1[:], pp[:], biasT[:, t:t + 1], None, ALU.add)
                        z2 = wp.tile([128, HALF], F32, name="z2", tag="z2")
                        nc.vector.tensor_scalar(
                            z2[:], z1[:], ALPHA, None, ALU.mult)
                        e_t = wp.tile([128, HALF], F32, name="e_t", tag="e")
                        nc.vector.tensor_tensor(e_t[:], z1[:], z2[:], ALU.max)
                    else:
                        # LeakyRelu on ACT with per-partition bias
                        e_t = wp.tile([128, HALF], F32, name="e_t", tag="e")
                        nc.scalar.activation(
                            e_t[:], pp[:], AF.Prelu,
                            bias=biasT[:, t:t + 1], scale=1.0, alpha=alpha02[:])
                    nc.scalar.activation(
                        pa[:], e_t[:], AF.Exp, bias=negS[:], scale=1.0)
                    # H^T accumulation (+ ones column -> row sums)
                    for c in range(2):
                        nc.tensor.matmul(
                            ht_ps[:, 512 * c:512 * (c + 1)],
                            whaug[:, (D + 1) * t:(D + 1) * (t + 1)],
                            pa[:, 512 * c:512 * (c + 1)],
                            start=(t == 0), stop=(t == NT - 1),
                        )
                # ---- epilogue for this half ----
                ht_sb = wp.tile([D + 1, HALF], F32, name="ht_sb", tag="ht_sb", bufs=2)
                nc.vector.tensor_copy(ht_sb[:], ht_ps[:])
                for k in range(8):
                    t2 = 8 * h + k
                    tr = psT.tile([128, D + 1], F32, name="tr", tag="tr")
                    nc.tensor.matmul(
                        tr[:], ht_sb[:, 128 * k:128 * (k + 1)], i65[:],
                        is_transpose=True, start=True, stop=True)
                    rc = wp.tile([128, 1], F32, name="rc", tag="rc", bufs=4)
                    nc.vector.reciprocal(rc[:], tr[:, D:D + 1])
                    # elu(x*r) = relu(x*r) + min(exp(x*r) - 1, 0)
                    w_t = wp.tile([128, D], F32, name="w_t", tag="w_t")
                    nc.scalar.activation(
                        w_t[:], tr[:, 0:D], AF.Exp, bias=0.0, scale=rc[:])
                    q_t = wp.tile([128, D], F32, name="q_t", tag="q_t")
                    nc.vector.tensor_scalar(
                        q_t[:], w_t[:], -1.0, 0.0, ALU.add, ALU.min)
                    r2 = wp.tile([128, D], F32, name="r2", tag="r2")
                    nc.vector.tensor_scalar(
                        r2[:], tr[:, 0:D], rc[:], 0.0, ALU.mult, ALU.max)
                    nc.vector.tensor_tensor(
                        hout[:, D * t2:D * (t2 + 1)], q_t[:], r2[:], ALU.add)
                nc.sync.dma_start(
                    H_d.ap()[1024 * h:1024 * (h + 1), :]
                    .rearrange("(t p) d -> p t d", p=128),
                    hout[:, 8 * D * h:8 * D * (h + 1)]
                    .rearrange("p (t d) -> p t d", d=D),
                )



    nc.compile()
    return nc


def _get_program():
    if "nc" not in _CACHED:
        _CACHED["nc"] = _build_program()
    return _CACHED["nc"]


def _host_prep(A, X, Ws, a):
    """Per-core host-side input preparation (cheap: ~67 MFLOP total)."""
    f64 = np.float64
    in_maps = []
    for b in range(B):
        Wh = X[b].astype(f64) @ Ws.astype(f64)            # [N, D]
        Wh1 = (Wh @ a[:D].astype(f64))[:, 0]              # [N]
        Wh2 = (Wh @ a[D:].astype(f64))[:, 0]              # [N]
        S = max(0.0, float(Wh1.max() + Wh2.max()) - 10.5)
        whaug = np.ones((N, D + 1), np.float16)
        whaug[:, :D] = Wh.astype(np.float16)
        wh1_hi = Wh1.astype(ml_dtypes.bfloat16)
        wh1_lo = (Wh1 - wh1_hi.astype(f64)).astype(ml_dtypes.bfloat16)
        wh1p = np.stack([wh1_hi, wh1_lo])                  # [2, N]
        biasT = (Wh2 - C_MASK).astype(np.float32).reshape(NT, 128).T.copy()
        in_maps.append({
            "A": np.ascontiguousarray(A[b]),
            "WhAug16": whaug,
            "wh1p": wh1p,
            "biasT": np.ascontiguousarray(biasT),
            "negS": np.full((128, 1), -S, np.float32),
            "CI": (C_MASK * np.eye(128)).astype(ml_dtypes.bfloat16),
            "ones2": np.ones((2, 128), ml_dtypes.bfloat16),
            "I65": np.eye(D + 1, dtype=np.float32),
        })
    return in_maps


def kernel(A, X, Ws, a, _trace=False, _trace_kwargs=None):
    A = np.asarray(A, np.float32)
    X = np.asarray(X, np.float32)
    Ws = np.asarray(Ws, np.float32)
    a = np.asarray(a, np.float32)
    nc = _get_program()
    in_maps = _host_prep(A, X, Ws, a)
    kw = {}
    if _trace:
        kw = {"trace": True, **(_trace_kwargs or {})}
    res = bass_utils.run_bass_kernel_spmd(nc, in_maps, core_ids=list(range(B)), **kw)
    H = np.stack([np.asarray(res.results[b]["H"]) for b in range(B)])
    if _trace:
        kernel.last_results = res
    return H



# revision 6
# speedup vs baseline: 1.2260x; 1.2260x over previous
"""GAT layer (dense adjacency) on 8 Trainium2 NeuronCores.

Problem: H = elu(softmax_j(mask(A, leaky_relu(Wh1_i + Wh2_j))) @ Wh),
A: [8, 2048, 2048] 0/1 f32, X: [8, 2048, 64], Ws: [64, 64], a: [128, 1].

Sharding: data-parallel over batch B=8 -> one batch element per core.

Single pass over 16 j-slabs (A column tiles), all 2048 i-columns per slab,
paced by the A DMA (~2.9 us per 1 MiB slab at 360 GB/s):
  - Host precomputes Wh = X@Ws, Wh1 = Wh@a1, Wh2 = Wh@a2 (tiny: 0.1% of work).
  - Per slab the masked logits are built in PSUM by the tensor engine:
        pp[j, i] = ones2^T @ [Wh1_hi; Wh1_lo]  (broadcast of Wh1 along j)
                 + (A_block)^T @ (C*I)         (mask: C=512 where edge, 0 else)
    (the matmul with C*I transposes A; no elementwise mask multiply needed).
  - LeakyRelu(pp + (Wh2[j]-C)) split ACT/DVE per slab: ACT Prelu for 768
    cols; DVE z1 = pp+bias (fp16), z2 = 0.2*z1 (4x mode), max(z1,z2)
    (2x mode) for the other 1280 cols.  ACT Exp(e - S) -> pa fp16.
  - H accumulates in natural [i, d] layout: per i-tile m,
    acc[:, col(m):col(m)+65] += pa[:, 128m:128(m+1)]^T @ WhAug  (fp16 inputs,
    ones column gives row sums); no output transpose needed.
  - Epilogue: rc = 1/s (batched strided reciprocals), y = acc*rc per tile
    (ACT Copy-scale / DVE TSP mix), then elu via the identity
    elu(y) = max(min(e^y - 1, 0), y):  W = Exp(y), t1 = min(W-1,0) (TSP),
    H = max(t1, y) (TT), two 512-col chunks, two output DMAs.
Engine budgets per slab (cost model): DMA 2913, ACT ~2900, DVE ~2830,
PE ~2130 ns -> DMA/ACT-bound steady state.
"""
import sys

for _p in ("/opt/trn_rl_repo",):
    if _p not in sys.path:
        sys.path.append(_p)

import numpy as np
import ml_dtypes

import concourse.bass as bass
import concourse.bacc as bacc
import concourse.tile as tile
from concourse import mybir
from concourse import bass_utils

F32 = mybir.dt.float32
BF16 = mybir.dt.bfloat16
FP16 = mybir.dt.float16
AF = mybir.ActivationFunctionType
ALU = mybir.AluOpType

B, N, F, D = 8, 2048, 64, 64
NT = N // 128          # 16 j-slabs / i-tiles
C_MASK = 512.0
ALPHA = 0.2
P_ACT = 768            # i-cols whose LeakyRelu runs on ACT (Prelu)
L_DVE = 2048 - P_ACT   # i-cols on the DVE z-chain

_CACHED = {}


def _acc_col(m):
    """Column offset of i-tile m inside the [128, 2048] PSUM accumulator.
    4 tiles per 512-f32 bank so no matmul output crosses a bank boundary."""
    return 512 * (m // 4) + 65 * (m % 4)


def _build_program():
    nc = bacc.Bacc("TRN2", target_bir_lowering=False, debug=False)

    A_d = nc.dram_tensor("A", [N, N], F32, kind="ExternalInput")
    whaug_d = nc.dram_tensor("WhAug16", [N, D + 1], FP16, kind="ExternalInput")
    wh1p_d = nc.dram_tensor("wh1p", [2, N], BF16, kind="ExternalInput")
    biasT_d = nc.dram_tensor("biasT", [128, NT], F32, kind="ExternalInput")
    negS_d = nc.dram_tensor("negS", [128, 1], F32, kind="ExternalInput")
    ci_d = nc.dram_tensor("CI", [128, 128], BF16, kind="ExternalInput")
    ones2_d = nc.dram_tensor("ones2", [2, 128], BF16, kind="ExternalInput")
    H_d = nc.dram_tensor("H", [N, D], F32, kind="ExternalOutput")

    with tile.TileContext(nc) as tc:
        with tc.tile_pool(name="const", bufs=1) as cp, \
             tc.tile_pool(name="aslab", bufs=8) as ap_pool, \
             tc.tile_pool(name="work", bufs=3) as wp, \
             tc.tile_pool(name="outp", bufs=1) as op_pool, \
             tc.tile_pool(name="psP", bufs=2, space="PSUM") as psP, \
             tc.tile_pool(name="psA", bufs=1, space="PSUM") as psA:

            # ---- constants / small inputs ----
            whaug = cp.tile([128, NT * (D + 1)], FP16, name="whaug")
            nc.sync.dma_start(
                whaug[:].rearrange("p (t c) -> p t c", c=D + 1),
                whaug_d.ap().rearrange("(t p) c -> p t c", p=128),
            )
            wh1p = cp.tile([2, N], BF16, name="wh1p")
            nc.sync.dma_start(wh1p[:], wh1p_d.ap())
            biasT = cp.tile([128, NT], F32, name="biasT")
            nc.sync.dma_start(biasT[:], biasT_d.ap())
            negS = cp.tile([128, 1], F32, name="negS")
            nc.sync.dma_start(negS[:], negS_d.ap())
            ci = cp.tile([128, 128], BF16, name="ci")
            nc.sync.dma_start(ci[:], ci_d.ap())
            ones2 = cp.tile([2, 128], BF16, name="ones2")
            nc.sync.dma_start(ones2[:], ones2_d.ap())
            alpha02 = cp.tile([128, 1], F32, name="alpha02")
            nc.vector.memset(alpha02[:], ALPHA)
            # preload the ACT table set during input DMA
            warm = cp.tile([1, 1], F32, name="warm")
            nc.vector.memset(warm[:], 0.0)
            warm2 = cp.tile([1, 1], F32, name="warm2")
            nc.scalar.activation(warm2[:], warm[:], AF.Exp, bias=0.0, scale=1.0)

            # H^pre accumulator (+ row sums), [i, d] layout, 4 banks.
            # Zeroed up front; the accumulating matmuls all use start=False
            # (a start=True resets the whole PSUM bank, which would wipe the
            # slab-0 contribution of the other 3 regions sharing the bank).
            acc = psA.tile([128, 2048], F32, name="acc")
            for b4 in range(4):
                nc.vector.memset(acc[:, 512 * b4:512 * (b4 + 1)], 0.0)

            aslabs = {}
            state = {}

            def dma_slab(t):
                sl = ap_pool.tile([128, N], F32, name=f"aslab{t}", tag="aslab")
                # sl[p, 128*r + q] = A[128*r + p, 128*t + q]
                nc.sync.dma_start(
                    sl[:].rearrange("p (r q) -> p r q", q=128),
                    A_d.ap()[:, 128 * t:128 * (t + 1)]
                    .rearrange("(r p) q -> p r q", p=128),
                )
                aslabs[t] = sl

            def fills(t):
                # pp_h[j, i] for i-halves h=0,1 (1024 cols each)
                pps = []
                for h in range(2):
                    pp = psP.tile([128, 1024], F32, name=f"pp{h}_{t}", tag="pp")
                    for c in range(2):
                        nc.tensor.matmul(
                            pp[:, 512 * c:512 * (c + 1)],
                            ones2[:],
                            wh1p[:, 1024 * h + 512 * c:1024 * h + 512 * (c + 1)],
                            start=True, stop=False,
                        )
                    for k8 in range(8):
                        r = 8 * h + k8
                        nc.tensor.matmul(
                            pp[:, 128 * k8:128 * (k8 + 1)],
                            aslabs[t][:].bitcast(BF16)[:, 256 * r + 1:256 * (r + 1):2],
                            ci[:],
                            start=False, stop=True, skip_group_check=True,
                        )
                    pps.append(pp)
                return pps

            def prelu_z1(t, pp0, pp1):
                # ACT Prelu for cols [0, 768); DVE z1 (fp16) for the rest
                e_act = wp.tile([128, P_ACT], F32, name=f"ea{t}", tag="ea")
                zt = wp.tile([128, L_DVE], FP16, name=f"zt{t}", tag="zt")
                b_ap = biasT[:, t:t + 1]
                nc.scalar.activation(
                    e_act[:], pp0[:, 0:P_ACT], AF.Prelu,
                    bias=b_ap, scale=1.0, alpha=alpha02[:])
                nc.vector.tensor_scalar(
                    zt[:, 0:256], pp0[:, 768:1024], b_ap, None, ALU.add)
                nc.vector.tensor_scalar(
                    zt[:, 256:768], pp1[:, 0:512], b_ap, None, ALU.add)
                nc.vector.tensor_scalar(
                    zt[:, 768:1280], pp1[:, 512:1024], b_ap, None, ALU.add)
                state[t] = {"ea": e_act, "zt": zt}

            def z2_tt(t):
                st = state[t]
                z2t = wp.tile([128, L_DVE], FP16, name=f"z2t{t}", tag="z2t")
                e_dve = wp.tile([128, L_DVE], FP16, name=f"ed{t}", tag="ed")
                nc.vector.tensor_scalar(
                    z2t[:], st["zt"][:], ALPHA, None, ALU.mult)
                nc.vector.tensor_tensor(e_dve[:], st["zt"][:], z2t[:], ALU.max)
                st["ed"] = e_dve

            def exp_act(t):
                st = state[t]
                pa = wp.tile([128, 2048], FP16, name=f"pa{t}", tag="pa", bufs=4)
                nc.scalar.activation(
                    pa[:, 0:P_ACT], st["ea"][:], AF.Exp, bias=negS[:], scale=1.0)
                st["pa"] = pa

            def exp_dve(t):
                st = state[t]
                nc.scalar.activation(
                    st["pa"][:, P_ACT:2048], st["ed"][:], AF.Exp,
                    bias=negS[:], scale=1.0)

            def accum(t):
                pa = state.pop(t)["pa"]
                for m in range(NT):
                    c0 = _acc_col(m)
                    nc.tensor.matmul(
                        acc[:, c0:c0 + D + 1],
                        pa[:, 128 * m:128 * (m + 1)],
                        whaug[:, (D + 1) * t:(D + 1) * (t + 1)],
                        start=False, stop=(t == NT - 1),
                        skip_group_check=True,
                    )

            # ---- software-pipelined main loop ----
            # Per-engine queue order keeps every queue head (nearly) always
            # ready: DVE runs lag-1 z2/max before this slab's z1; ACT runs
            # lag-1/lag-2 Exps before this slab's Prelu; PE runs fills before
            # the lag-3 accumulation (whose pa completed two iters ago).
            for t in range(4):
                dma_slab(t)
            for k in range(NT + 3):
                if k + 4 < NT:
                    dma_slab(k + 4)
                if 1 <= k <= NT:
                    z2_tt(k - 1)
                    exp_act(k - 1)
                if 2 <= k <= NT + 1:
                    exp_dve(k - 2)
                if k < NT:
                    pp0, pp1 = fills(k)
                    prelu_z1(k, pp0, pp1)
                if k >= 3:
                    accum(k - 3)

            # ---- epilogue: H = elu(acc*rc) = max(min(e^y - 1, 0), y) ----
            rc_all = op_pool.tile([128, NT], F32, name="rc_all")
            y_all = op_pool.tile([128, 1024], FP16, name="y_all")
            W_all = op_pool.tile([128, 1024], FP16, name="W_all")
            t1_all = op_pool.tile([128, 1024], FP16, name="t1_all")
            hout = op_pool.tile([128, 1024], F32, name="hout")
            for b in range(4):
                s0 = 512 * b + D
                nc.vector.reciprocal(
                    rc_all[:, 4 * b:4 * b + 4], acc[:, s0:s0 + 4 * 65:65])
            for half in range(2):
                for m in range(8 * half, 8 * half + 8):
                    c0 = _acc_col(m)
                    rc = rc_all[:, m:m + 1]
                    y_sl = y_all[:, D * m:D * (m + 1)]
                    if m % 3 == 0 or m == 14:
                        nc.vector.tensor_scalar(
                            y_sl, acc[:, c0:c0 + D], rc, None, ALU.mult)
                    else:
                        nc.scalar.activation(
                            y_sl, acc[:, c0:c0 + D], AF.Copy, bias=0.0, scale=rc)
                h0 = 512 * half
                nc.scalar.activation(
                    W_all[:, h0:h0 + 512], y_all[:, h0:h0 + 512],
                    AF.Exp, bias=0.0, scale=1.0)
                nc.vector.tensor_scalar(
                    t1_all[:, h0:h0 + 512], W_all[:, h0:h0 + 512],
                    -1.0, 0.0, ALU.add, ALU.min)
                nc.vector.tensor_tensor(
                    hout[:, h0:h0 + 512], t1_all[:, h0:h0 + 512],
                    y_all[:, h0:h0 + 512], ALU.max)
                nc.sync.dma_start(
                    H_d.ap()[1024 * half:1024 * (half + 1), :]
                    .rearrange("(t p) d -> p t d", p=128),
                    hout[:, h0:h0 + 512]
                    .rearrange("p (t d) -> p t d", d=D),
                )

    nc.compile()
    return nc


def _get_program():
    if "nc" not in _CACHED:
        _CACHED["nc"] = _build_program()
    return _CACHED["nc"]


def _host_prep(A, X, Ws, a):
    """Per-core host-side input preparation (cheap: ~67 MFLOP total)."""
    f64 = np.float64
    in_maps = []
    for b in range(B):
        Wh = X[b].astype(f64) @ Ws.astype(f64)            # [N, D]
        Wh1 = (Wh @ a[:D].astype(f64))[:, 0]              # [N]
        Wh2 = (Wh @ a[D:].astype(f64))[:, 0]              # [N]
        S = max(0.0, float(Wh1.max() + Wh2.max()) - 10.5)
        whaug = np.ones((N, D + 1), np.float16)
        whaug[:, :D] = Wh.astype(np.float16)
        wh1_hi = Wh1.astype(ml_dtypes.bfloat16)
        wh1_lo = (Wh1 - wh1_hi.astype(f64)).astype(ml_dtypes.bfloat16)
        wh1p = np.stack([wh1_hi, wh1_lo])                  # [2, N]
        biasT = (Wh2 - C_MASK).astype(np.float32).reshape(NT, 128).T.copy()
        in_maps.append({
            "A": np.ascontiguousarray(A[b]),
            "WhAug16": whaug,
            "wh1p": wh1p,
            "biasT": np.ascontiguousarray(biasT),
            "negS": np.full((128, 1), -S, np.float32),
            "CI": (C_MASK * np.eye(128)).astype(ml_dtypes.bfloat16),
            "ones2": np.ones((2, 128), ml_dtypes.bfloat16),
        })
    return in_maps


def kernel(A, X, Ws, a, _trace=False, _trace_kwargs=None):
    A = np.asarray(A, np.float32)
    X = np.asarray(X, np.float32)
    Ws = np.asarray(Ws, np.float32)
    a = np.asarray(a, np.float32)
    nc = _get_program()
    in_maps = _host_prep(A, X, Ws, a)
    kw = {}
    if _trace:
        kw = {"trace": True, **(_trace_kwargs or {})}
    res = bass_utils.run_bass_kernel_spmd(nc, in_maps, core_ids=list(range(B)), **kw)
    H = np.stack([np.asarray(res.results[b]["H"]) for b in range(B)])
    if _trace:
        kernel.last_results = res
    return H


# revision 8
# speedup vs baseline: 1.3701x; 1.1175x over previous
"""GAT layer (dense adjacency) on 8 Trainium2 NeuronCores.

Problem: H = elu(softmax_j(mask(A, leaky_relu(Wh1_i + Wh2_j))) @ Wh),
A: [8, 2048, 2048] 0/1 f32, X: [8, 2048, 64], Ws: [64, 64], a: [128, 1].

Sharding: data-parallel over batch B=8 -> one batch element per core.

Single pass over 16 j-slabs (A column tiles), all 2048 i-columns per slab,
paced by the A DMA (~2.9 us per 1 MiB slab at 360 GB/s):
  - Host precomputes Wh = X@Ws, Wh1 = Wh@a1, Wh2 = Wh@a2 (tiny: 0.1% of
    work) and packs all small inputs into two DMA blobs.
  - Per slab the masked logits are built in PSUM (4 single-bank chunks of
    512 i-cols, ring of 5 banks) by the tensor engine:
        pp[j, i] = ones2^T @ [Wh1_hi; Wh1_lo]  (broadcast of Wh1 along j)
                 + (A_block)^T @ (C*I)         (mask: C=512 where edge, 0 else)
    (the matmul with C*I transposes A; no elementwise mask multiply needed).
  - LeakyRelu(pp + (Wh2[j]-C)) split ACT/DVE per slab: ACT Prelu for 672
    cols; DVE z1 = pp+bias (fp16), z2 = 0.2*z1 (4x mode), max(z1,z2)
    (2x mode) for the other 1376 cols.  ACT Exp(e - S) -> pa fp16.
  - H accumulates in natural [i, d] layout: per i-tile m,
    acc[:, col(m):col(m)+65] += pa[:, 128m:128(m+1)]^T @ WhAug  (fp16
    inputs, ones column gives row sums).  The accumulator is zeroed once
    and all matmuls use start=False (start=True resets the whole PSUM
    bank, which would wipe co-resident regions).  3 banks, 7/7/2 packing.
  - Epilogue (batched, no per-tile ops): strided reciprocals -> rc;
    y = acc * broadcast(rc) (one TT per bank group); elu via the identity
    elu(y) = max(min(e^y - 1, 0), y):  W = Exp(y), t1 = min(W-1,0) (TSP),
    H = max(t1, y) (TT), in two 512-col chunks, two output DMAs.
"""
import sys

for _p in ("/opt/trn_rl_repo",):
    if _p not in sys.path:
        sys.path.append(_p)

import numpy as np
import ml_dtypes

import concourse.bass as bass
import concourse.bacc as bacc
import concourse.tile as tile
from concourse import mybir
from concourse import bass_utils

F32 = mybir.dt.float32
BF16 = mybir.dt.bfloat16
FP16 = mybir.dt.float16
AF = mybir.ActivationFunctionType
ALU = mybir.AluOpType

B, N, F, D = 8, 2048, 64, 64
NT = N // 128          # 16 j-slabs / i-tiles
C_MASK = 512.0
ALPHA = 0.2
P_ACT = 672            # i-cols whose LeakyRelu runs on ACT (Prelu)
L_DVE = 2048 - P_ACT   # i-cols on the DVE z-chain (1376)
# blob1 f32 column layout
B1_WH, B1_BIAS, B1_NEGS, B1_CI, B1_W = 0, 520, 536, 537, 601

_CACHED = {}


def _acc_col(m):
    """Column offset of i-tile m inside the [128, 1536] PSUM accumulator
    (7 tiles in bank 0, 7 in bank 1, 2 in bank 2)."""
    return 512 * (m // 7) + 65 * (m % 7)


def _build_program():
    nc = bacc.Bacc("TRN2", target_bir_lowering=False, debug=False)

    A_d = nc.dram_tensor("A", [N, N], F32, kind="ExternalInput")
    blob1_d = nc.dram_tensor("blob1", [128, B1_W], F32, kind="ExternalInput")
    blob2_d = nc.dram_tensor("blob2", [2, 2176], BF16, kind="ExternalInput")
    H_d = nc.dram_tensor("H", [N, D], F32, kind="ExternalOutput")

    with tile.TileContext(nc) as tc:
        with tc.tile_pool(name="const", bufs=1) as cp, \
             tc.tile_pool(name="aslab", bufs=8) as ap_pool, \
             tc.tile_pool(name="work", bufs=3) as wp, \
             tc.tile_pool(name="outp", bufs=1) as op_pool, \
             tc.tile_pool(name="psP", bufs=5, space="PSUM") as psP, \
             tc.tile_pool(name="psA", bufs=1, space="PSUM") as psA:

            # ---- consolidated constants (2 DMAs) ----
            cb1 = cp.tile([128, B1_W], F32, name="cb1")
            nc.sync.dma_start(cb1[:], blob1_d.ap())
            cb2 = cp.tile([2, 2176], BF16, name="cb2")
            nc.sync.dma_start(cb2[:], blob2_d.ap())
            whaug = cb1[:].bitcast(FP16)[:, 0:1040]          # [128, 16*65]
            biasT = cb1[:, B1_BIAS:B1_BIAS + NT]
            negS = cb1[:, B1_NEGS:B1_NEGS + 1]
            ci = cb1[:].bitcast(BF16)[:, 2 * B1_CI:2 * B1_CI + 128]
            wh1p = cb2[:, 0:2048]
            ones2 = cb2[:, 2048:2176]
            alpha02 = cp.tile([128, 1], F32, name="alpha02")
            nc.vector.memset(alpha02[:], ALPHA)
            # preload the ACT table set during input DMA
            warm = cp.tile([1, 1], F32, name="warm")
            nc.vector.memset(warm[:], 0.0)
            warm2 = cp.tile([1, 1], F32, name="warm2")
            nc.scalar.activation(warm2[:], warm[:], AF.Exp, bias=0.0, scale=1.0)

            # H^pre accumulator (+ row sums), [i, d] layout, 3 banks.
            # Zeroed once; the accumulating matmuls all use start=False
            # (start=True resets the whole PSUM bank, wiping the slab-0
            # contribution of co-resident regions).
            acc = psA.tile([128, 1536], F32, name="acc")
            for b3 in range(3):
                nc.vector.memset(acc[:, 512 * b3:512 * (b3 + 1)], 0.0)

            aslabs = {}
            state = {}

            def dma_slab(t, split=False):
                sl = ap_pool.tile([128, N], F32, name=f"aslab{t}", tag="aslab")
                # sl[p, 128*r + q] = A[128*r + p, 128*t + q]
                if split:  # 4 row-block chunks so fills can start early
                    for rr in range(4):
                        nc.sync.dma_start(
                            sl[:, 512 * rr:512 * (rr + 1)]
                            .rearrange("p (r q) -> p r q", q=128),
                            A_d.ap()[512 * rr:512 * (rr + 1),
                                     128 * t:128 * (t + 1)]
                            .rearrange("(r p) q -> p r q", p=128),
                        )
                else:
                    nc.sync.dma_start(
                        sl[:].rearrange("p (r q) -> p r q", q=128),
                        A_d.ap()[:, 128 * t:128 * (t + 1)]
                        .rearrange("(r p) q -> p r q", p=128),
                    )
                aslabs[t] = sl

            def fills(t):
                # 4 single-bank chunks of 512 i-cols each
                chunks = []
                for c in range(4):
                    pp = psP.tile([128, 512], F32, name=f"pp{c}_{t}", tag="pp")
                    nc.tensor.matmul(
                        pp[:], ones2[:], wh1p[:, 512 * c:512 * (c + 1)],
                        start=True, stop=False,
                    )
                    for k4 in range(4):
                        r = 4 * c + k4
                        nc.tensor.matmul(
                            pp[:, 128 * k4:128 * (k4 + 1)],
                            aslabs[t][:].bitcast(BF16)[:, 256 * r + 1:256 * (r + 1):2],
                            ci[:],
                            start=False, stop=True, skip_group_check=True,
                        )
                    chunks.append(pp)
                return chunks

            def prelu_z1(t, pp):
                # ACT Prelu for cols [0, 672); DVE z1 (fp16) for the rest
                e_act = wp.tile([128, P_ACT], F32, name=f"ea{t}", tag="ea")
                zt = wp.tile([128, L_DVE], FP16, name=f"zt{t}", tag="zt")
                b_ap = biasT[:, t:t + 1]
                nc.scalar.activation(
                    e_act[:, 0:512], pp[0][:], AF.Prelu,
                    bias=b_ap, scale=1.0, alpha=alpha02[:])
                nc.scalar.activation(
                    e_act[:, 512:P_ACT], pp[1][:, 0:P_ACT - 512], AF.Prelu,
                    bias=b_ap, scale=1.0, alpha=alpha02[:])
                nc.vector.tensor_scalar(
                    zt[:, 0:352], pp[1][:, 160:512], b_ap, None, ALU.add)
                nc.vector.tensor_scalar(
                    zt[:, 352:864], pp[2][:], b_ap, None, ALU.add)
                nc.vector.tensor_scalar(
                    zt[:, 864:1376], pp[3][:], b_ap, None, ALU.add)
                state[t] = {"ea": e_act, "zt": zt}

            def z2_tt(t, parts):
                # parts: list of (start, end) ranges within zt
                st = state[t]
                st["ed"] = []
                for (s0, s1) in parts:
                    z2t = wp.tile([128, s1 - s0], FP16, name=f"z2t{t}_{s0}",
                                  tag="z2t" if len(parts) == 1 else f"z2s{s0}",
                                  bufs=3 if len(parts) == 1 else 1)
                    e_dve = wp.tile([128, s1 - s0], FP16, name=f"ed{t}_{s0}",
                                    tag="ed" if len(parts) == 1 else f"eds{s0}",
                                    bufs=3 if len(parts) == 1 else 1)
                    nc.vector.tensor_scalar(
                        z2t[:], st["zt"][:, s0:s1], ALPHA, None, ALU.mult)
                    nc.vector.tensor_tensor(
                        e_dve[:], st["zt"][:, s0:s1], z2t[:], ALU.max)
                    st["ed"].append((s0, s1, e_dve))

            def exp_act(t, pa_tiles):
                st = state[t]
                st["pa"] = pa_tiles
                nc.scalar.activation(
                    pa_tiles[0][:, 0:P_ACT], st["ea"][:], AF.Exp,
                    bias=negS, scale=1.0)

            def exp_dve(t):
                st = state[t]
                for (s0, s1, e_dve) in st["ed"]:
                    c0, c1 = P_ACT + s0, P_ACT + s1
                    if c0 >= 1024:
                        tgt = st["pa"][-1][:, c0 - 1024:c1 - 1024]
                    else:
                        tgt = st["pa"][0][:, c0:c1]
                    nc.scalar.activation(
                        tgt, e_dve[:], AF.Exp, bias=negS, scale=1.0)

            def accum(t):
                pa_tiles = state.pop(t)["pa"]
                for m in range(NT):
                    c0 = _acc_col(m)
                    if len(pa_tiles) == 1:
                        lhs = pa_tiles[0][:, 128 * m:128 * (m + 1)]
                    else:
                        lhs = pa_tiles[m // 8][:, 128 * (m % 8):128 * (m % 8 + 1)]
                    nc.tensor.matmul(
                        acc[:, c0:c0 + D + 1],
                        lhs,
                        whaug[:, (D + 1) * t:(D + 1) * (t + 1)],
                        start=False, stop=(t == NT - 1),
                        skip_group_check=True,
                    )

            # ---- software-pipelined main loop ----
            # Lag structure keeps every engine queue head (nearly) always
            # ready: DVE runs lag-1 z2/max after this slab's z1 feeds in;
            # ACT runs lag-1/lag-2 Exps before this slab's Prelu; PE runs
            # fills before the lag-3 accumulation.  The last slab (t=15)
            # splits its z2/max/Exp into halves with separate pa tiles so
            # the drain pipelines.
            LAST = NT - 1
            dma_slab(0, split=True)
            for t in range(1, 4):
                dma_slab(t)
            for k in range(NT + 3):
                if k + 4 < NT:
                    dma_slab(k + 4)
                if 1 <= k <= NT:
                    t = k - 1
                    if t == LAST:
                        z2_tt(t, [(0, 352), (352, L_DVE)])
                    else:
                        z2_tt(t, [(0, L_DVE)])
                    if t == LAST:
                        pa_tiles = [
                            wp.tile([128, 1024], FP16, name="paL0", bufs=1),
                            wp.tile([128, 1024], FP16, name="paL1", bufs=1),
                        ]
                    else:
                        pa_tiles = [wp.tile([128, 2048], FP16,
                                            name=f"pa{t}", tag="pa", bufs=4)]
                    exp_act(t, pa_tiles)
                if 2 <= k <= NT + 1:
                    exp_dve(k - 2)
                if k < NT:
                    pp = fills(k)
                    prelu_z1(k, pp)
                if k >= 3:
                    accum(k - 3)

            # ---- epilogue: H = elu(acc*rc) = max(min(e^y - 1, 0), y) ----
            rc_all = op_pool.tile([128, NT], F32, name="rc_all")
            y_all = op_pool.tile([128, 1024], FP16, name="y_all")
            W_all = op_pool.tile([128, 1024], FP16, name="W_all")
            t1_all = op_pool.tile([128, 1024], FP16, name="t1_all")
            hout = op_pool.tile([128, 1024], F32, name="hout")

            accv = acc[:].rearrange("p (b q) -> p b q", b=3)
            # reciprocals of the row sums (strided PSUM reads)
            nc.vector.reciprocal(
                rc_all[:, 0:14].rearrange("p (b g) -> p b g", b=2),
                accv[:, 0:2, 64:455:65])
            nc.vector.reciprocal(
                rc_all[:, 14:16], acc[:, 1088:1218:65])
            # y = H_pre * rc, batched via broadcast views
            hp01 = accv[:, 0:2, 0:455] \
                .rearrange("p b (g c) -> p b g c", c=65)[:, :, :, 0:64]
            rc01 = rc_all[:, 0:14].rearrange("p (b g) -> p b g", b=2) \
                .unsqueeze(3).broadcast_to([128, 2, 7, 64])
            nc.vector.tensor_tensor(
                y_all[:, 0:896].rearrange("p (b g c) -> p b g c", b=2, c=64),
                hp01, rc01, ALU.mult)
            hp2 = acc[:, 1024:1154] \
                .rearrange("p (g c) -> p g c", c=65)[:, :, 0:64]
            rc2 = rc_all[:, 14:16].rearrange("p g -> p g") \
                .unsqueeze(2).broadcast_to([128, 2, 64])
            nc.vector.tensor_tensor(
                y_all[:, 896:1024].rearrange("p (g c) -> p g c", c=64),
                hp2, rc2, ALU.mult)
            for half in range(2):
                h0 = 512 * half
                nc.scalar.activation(
                    W_all[:, h0:h0 + 512], y_all[:, h0:h0 + 512],
                    AF.Exp, bias=0.0, scale=1.0)
                nc.vector.tensor_scalar(
                    t1_all[:, h0:h0 + 512], W_all[:, h0:h0 + 512],
                    -1.0, 0.0, ALU.add, ALU.min)
                nc.vector.tensor_tensor(
                    hout[:, h0:h0 + 512], t1_all[:, h0:h0 + 512],
                    y_all[:, h0:h0 + 512], ALU.max)
                nc.sync.dma_start(
                    H_d.ap()[1024 * half:1024 * (half + 1), :]
                    .rearrange("(t p) d -> p t d", p=128),
                    hout[:, h0:h0 + 512]
                    .rearrange("p (t d) -> p t d", d=D),
                )

    nc.compile()
    return nc


def _get_program():
    if "nc" not in _CACHED:
        _CACHED["nc"] = _build_program()
    return _CACHED["nc"]


def _host_prep(A, X, Ws, a):
    """Per-core host-side input preparation (cheap: ~67 MFLOP total)."""
    f64 = np.float64
    in_maps = []
    ci = (C_MASK * np.eye(128)).astype(ml_dtypes.bfloat16)
    for b in range(B):
        Wh = X[b].astype(f64) @ Ws.astype(f64)            # [N, D]
        Wh1 = (Wh @ a[:D].astype(f64))[:, 0]              # [N]
        Wh2 = (Wh @ a[D:].astype(f64))[:, 0]              # [N]
        S = max(0.0, float(Wh1.max() + Wh2.max()) - 10.5)
        whaug = np.ones((N, D + 1), np.float16)
        whaug[:, :D] = Wh.astype(np.float16)
        wh1_hi = Wh1.astype(ml_dtypes.bfloat16)
        wh1_lo = (Wh1 - wh1_hi.astype(f64)).astype(ml_dtypes.bfloat16)
        biasT = (Wh2 - C_MASK).astype(np.float32).reshape(NT, 128).T

        blob1 = np.zeros((128, B1_W), np.float32)
        b1u = blob1.view(np.uint16)
        b1u[:, 0:1040] = whaug.reshape(NT, 128, D + 1) \
            .transpose(1, 0, 2).reshape(128, 1040).view(np.uint16)
        blob1[:, B1_BIAS:B1_BIAS + NT] = biasT
        blob1[:, B1_NEGS] = -S
        b1u[:, 2 * B1_CI:2 * B1_CI + 128] = ci.view(np.uint16)

        blob2 = np.zeros((2, 2176), ml_dtypes.bfloat16)
        blob2[0, 0:2048] = wh1_hi
        blob2[1, 0:2048] = wh1_lo
        blob2[:, 2048:2176] = np.ones((2, 128), ml_dtypes.bfloat16)

        in_maps.append({
            "A": np.ascontiguousarray(A[b]),
            "blob1": blob1,
            "blob2": blob2,
        })
    return in_maps


def kernel(A, X, Ws, a, _trace=False, _trace_kwargs=None):
    A = np.asarray(A, np.float32)
    X = np.asarray(X, np.float32)
    Ws = np.asarray(Ws, np.float32)
    a = np.asarray(a, np.float32)
    nc = _get_program()
    in_maps = _host_prep(A, X, Ws, a)
    kw = {}
    if _trace:
        kw = {"trace": True, **(_trace_kwargs or {})}
    res = bass_utils.run_bass_kernel_spmd(nc, in_maps, core_ids=list(range(B)), **kw)
    H = np.stack([np.asarray(res.results[b]["H"]) for b in range(B)])
    if _trace:
        kernel.last_results = res
    return H


# revision 9
# speedup vs baseline: 1.4335x; 1.0463x over previous
"""GAT layer (dense adjacency) on 8 Trainium2 NeuronCores.

Problem: H = elu(softmax_j(mask(A, leaky_relu(Wh1_i + Wh2_j))) @ Wh),
A: [8, 2048, 2048] 0/1 f32, X: [8, 2048, 64], Ws: [64, 64], a: [128, 1].

Sharding: data-parallel over batch B=8 -> one batch element per core.

Single pass over 16 j-slabs (A column tiles), all 2048 i-columns per slab,
paced by the A DMA (~2.9 us per 1 MiB slab at 360 GB/s):
  - Host precomputes Wh = X@Ws, Wh1 = Wh@a1, Wh2 = Wh@a2 (tiny: 0.1% of
    work) and packs the small inputs into three DMA blobs (small consts
    first so the first slab's compute starts early).
  - Per slab the masked logits are built in PSUM (4 single-bank chunks of
    512 i-cols, ring of 5 banks) by the tensor engine:
        pp[j, i] = ones2^T @ [Wh1_hi; Wh1_lo]  (broadcast of Wh1 along j)
                 + (A_block)^T @ (C*I)         (mask: C=512 where edge, 0 else)
    (the matmul with C*I transposes A; no elementwise mask multiply needed).
  - LeakyRelu(pp + (Wh2[j]-C)) split ACT/DVE per slab: ACT Prelu for 672
    cols; DVE z1 = pp+bias (fp16), z2 = 0.2*z1 (4x mode), max(z1,z2)
    (2x mode) for the other 1376 cols.  ACT Exp(e - S) -> pa fp16.
  - H accumulates in natural [i, d] layout: per i-tile m,
    acc[:, col(m):col(m)+65] += pa[:, 128m:128(m+1)]^T @ WhAug  (fp16
    inputs, ones column gives row sums).  The accumulator is zeroed once
    and all matmuls use start=False (start=True resets the whole PSUM
    bank, which would wipe co-resident regions).  3 banks, 7/7/2 packing.
  - The last slab splits its z2/max/Exp into three pa pieces aligned with
    the accumulator banks, so each bank's epilogue chain starts as soon
    as its own data is complete.
  - Epilogue per bank (batched, no per-tile ops): strided reciprocals ->
    rc; y = acc * broadcast(rc) (one TT); elu via the identity
    elu(y) = max(min(e^y - 1, 0), y):  W = Exp(y), t1 = min(W-1,0) (TSP),
    H = max(t1, y) (TT, fp16), one output DMA per bank.  H is written
    fp16 and cast to f32 on the host (5e-4 quantization, halves the
    output DMA).
"""
import sys

for _p in ("/opt/trn_rl_repo",):
    if _p not in sys.path:
        sys.path.append(_p)

import numpy as np
import ml_dtypes

import concourse.bass as bass
import concourse.bacc as bacc
import concourse.tile as tile
from concourse import mybir
from concourse import bass_utils

F32 = mybir.dt.float32
BF16 = mybir.dt.bfloat16
FP16 = mybir.dt.float16
AF = mybir.ActivationFunctionType
ALU = mybir.AluOpType

B, N, F, D = 8, 2048, 64, 64
NT = N // 128          # 16 j-slabs / i-tiles
C_MASK = 512.0
ALPHA = 0.2
P_ACT = 672            # i-cols whose LeakyRelu runs on ACT (Prelu)
L_DVE = 2048 - P_ACT   # i-cols on the DVE z-chain (1376)
# blobS f32 column layout: biasT | negS | pad | ci(bf16)
BS_BIAS, BS_NEGS, BS_CI, BS_W = 0, 16, 18, 82

_CACHED = {}


def _acc_col(m):
    """Column offset of i-tile m inside the [128, 1536] PSUM accumulator
    (7 tiles in bank 0, 7 in bank 1, 2 in bank 2)."""
    return 512 * (m // 7) + 65 * (m % 7)


def _build_program():
    nc = bacc.Bacc("TRN2", target_bir_lowering=False, debug=False)

    A_d = nc.dram_tensor("A", [N, N], F32, kind="ExternalInput")
    blobS_d = nc.dram_tensor("blobS", [128, BS_W], F32, kind="ExternalInput")
    blobW_d = nc.dram_tensor("blobW", [128, 520], F32, kind="ExternalInput")
    blob2_d = nc.dram_tensor("blob2", [2, 2176], BF16, kind="ExternalInput")
    H_d = nc.dram_tensor("H", [N, D], FP16, kind="ExternalOutput")

    with tile.TileContext(nc) as tc:
        with tc.tile_pool(name="const", bufs=1) as cp, \
             tc.tile_pool(name="aslab", bufs=8) as ap_pool, \
             tc.tile_pool(name="work", bufs=3) as wp, \
             tc.tile_pool(name="outp", bufs=1) as op_pool, \
             tc.tile_pool(name="psP", bufs=5, space="PSUM") as psP, \
             tc.tile_pool(name="psA", bufs=1, space="PSUM") as psA:

            # ---- constants (3 DMAs; small ones first) ----
            cbS = cp.tile([128, BS_W], F32, name="cbS")
            nc.sync.dma_start(cbS[:], blobS_d.ap())
            cb2 = cp.tile([2, 2176], BF16, name="cb2")
            nc.sync.dma_start(cb2[:], blob2_d.ap())
            cbW = cp.tile([128, 520], F32, name="cbW")
            biasT = cbS[:, BS_BIAS:BS_BIAS + NT]
            negS = cbS[:, BS_NEGS:BS_NEGS + 1]
            ci = cbS[:].bitcast(BF16)[:, 2 * BS_CI:2 * BS_CI + 128]
            wh1p = cb2[:, 0:2048]
            ones2 = cb2[:, 2048:2176]
            whaug = cbW[:].bitcast(FP16)[:, 0:1040]          # [128, 16*65]
            alpha02 = cp.tile([128, 1], F32, name="alpha02")
            nc.vector.memset(alpha02[:], ALPHA)
            # preload the ACT table set during input DMA
            warm = cp.tile([1, 1], F32, name="warm")
            nc.vector.memset(warm[:], 0.0)
            warm2 = cp.tile([1, 1], F32, name="warm2")
            nc.scalar.activation(warm2[:], warm[:], AF.Exp, bias=0.0, scale=1.0)

            # H^pre accumulator (+ row sums), [i, d] layout, 3 banks.
            # Zeroed once; the accumulating matmuls all use start=False
            # (start=True resets the whole PSUM bank, wiping the slab-0
            # contribution of co-resident regions).
            acc = psA.tile([128, 1536], F32, name="acc")
            for b3 in range(3):
                nc.vector.memset(acc[:, 512 * b3:512 * (b3 + 1)], 0.0)

            aslabs = {}
            state = {}

            def dma_slab(t, split=False):
                sl = ap_pool.tile([128, N], F32, name=f"aslab{t}", tag="aslab")
                # sl[p, 128*r + q] = A[128*r + p, 128*t + q]
                if split:  # 4 row-block chunks so fills can start early
                    for rr in range(4):
                        nc.sync.dma_start(
                            sl[:, 512 * rr:512 * (rr + 1)]
                            .rearrange("p (r q) -> p r q", q=128),
                            A_d.ap()[512 * rr:512 * (rr + 1),
                                     128 * t:128 * (t + 1)]
                            .rearrange("(r p) q -> p r q", p=128),
                        )
                else:
                    nc.sync.dma_start(
                        sl[:].rearrange("p (r q) -> p r q", q=128),
                        A_d.ap()[:, 128 * t:128 * (t + 1)]
                        .rearrange("(r p) q -> p r q", p=128),
                    )
                aslabs[t] = sl

            def fills(t):
                # 4 single-bank chunks of 512 i-cols each
                chunks = []
                for c in range(4):
                    pp = psP.tile([128, 512], F32, name=f"pp{c}_{t}", tag="pp")
                    nc.tensor.matmul(
                        pp[:], ones2[:], wh1p[:, 512 * c:512 * (c + 1)],
                        start=True, stop=False,
                    )
                    for k4 in range(4):
                        r = 4 * c + k4
                        nc.tensor.matmul(
                            pp[:, 128 * k4:128 * (k4 + 1)],
                            aslabs[t][:].bitcast(BF16)[:, 256 * r + 1:256 * (r + 1):2],
                            ci[:],
                            start=False, stop=True, skip_group_check=True,
                        )
                    chunks.append(pp)
                return chunks

            def prelu_z1(t, pp):
                # ACT Prelu for cols [0, 672); DVE z1 (fp16) for the rest
                e_act = wp.tile([128, P_ACT], F32, name=f"ea{t}", tag="ea")
                zt = wp.tile([128, L_DVE], FP16, name=f"zt{t}", tag="zt")
                b_ap = biasT[:, t:t + 1]
                nc.scalar.activation(
                    e_act[:, 0:512], pp[0][:], AF.Prelu,
                    bias=b_ap, scale=1.0, alpha=alpha02[:])
                nc.scalar.activation(
                    e_act[:, 512:P_ACT], pp[1][:, 0:P_ACT - 512], AF.Prelu,
                    bias=b_ap, scale=1.0, alpha=alpha02[:])
                nc.vector.tensor_scalar(
                    zt[:, 0:352], pp[1][:, 160:512], b_ap, None, ALU.add)
                nc.vector.tensor_scalar(
                    zt[:, 352:864], pp[2][:], b_ap, None, ALU.add)
                nc.vector.tensor_scalar(
                    zt[:, 864:1376], pp[3][:], b_ap, None, ALU.add)
                state[t] = {"ea": e_act, "zt": zt}

            def z2_tt(t, parts):
                # parts: list of (start, end) ranges within zt
                st = state[t]
                st["ed"] = []
                for (s0, s1) in parts:
                    one = len(parts) == 1
                    z2t = wp.tile([128, s1 - s0], FP16, name=f"z2t{t}_{s0}",
                                  tag="z2t" if one else f"z2s{s0}",
                                  bufs=3 if one else 1)
                    e_dve = wp.tile([128, s1 - s0], FP16, name=f"ed{t}_{s0}",
                                    tag="ed" if one else f"eds{s0}",
                                    bufs=3 if one else 1)
                    nc.vector.tensor_scalar(
                        z2t[:], st["zt"][:, s0:s1], ALPHA, None, ALU.mult)
                    nc.vector.tensor_tensor(
                        e_dve[:], st["zt"][:, s0:s1], z2t[:], ALU.max)
                    st["ed"].append((s0, s1, e_dve))

            def exp_act(t, pa_map):
                # pa_map: list of (col0, col1, tile); ExpA writes [0, P_ACT)
                st = state[t]
                st["pa_map"] = pa_map
                c0, c1, tl = pa_map[0]
                assert c0 == 0 and c1 >= P_ACT
                nc.scalar.activation(
                    tl[:, 0:P_ACT], st["ea"][:], AF.Exp, bias=negS, scale=1.0)

            def _pa_slice(pa_map, c0, c1):
                for (p0, p1, tl) in pa_map:
                    if p0 <= c0 and c1 <= p1:
                        return tl[:, c0 - p0:c1 - p0]
                raise AssertionError((c0, c1))

            def exp_dve(t):
                st = state[t]
                for (s0, s1, e_dve) in st["ed"]:
                    tgt = _pa_slice(st["pa_map"], P_ACT + s0, P_ACT + s1)
                    nc.scalar.activation(
                        tgt, e_dve[:], AF.Exp, bias=negS, scale=1.0)

            def accum(t):
                pa_map = state.pop(t)["pa_map"]
                for m in range(NT):
                    c0 = _acc_col(m)
                    nc.tensor.matmul(
                        acc[:, c0:c0 + D + 1],
                        _pa_slice(pa_map, 128 * m, 128 * (m + 1)),
                        whaug[:, (D + 1) * t:(D + 1) * (t + 1)],
                        start=False, stop=(t == NT - 1),
                        skip_group_check=True,
                    )

            # ---- software-pipelined main loop ----
            # Lag structure keeps every engine queue head (nearly) always
            # ready: DVE runs lag-1 z2/max; ACT runs lag-1/lag-2 Exps before
            # this slab's Prelu; PE runs fills before the lag-3 accumulation.
            # The last slab (t=15) splits into three pa pieces aligned with
            # the accumulator banks (tiles 0-7 / 8-13 / 14-15) so the
            # per-bank epilogue chains start as early as possible.
            LAST = NT - 1
            dma_slab(0, split=True)
            whaug_dma_done = False
            for t in range(1, 4):
                dma_slab(t)
            nc.sync.dma_start(cbW[:], blobW_d.ap())
            for k in range(NT + 3):
                if k + 4 < NT:
                    dma_slab(k + 4)
                if 1 <= k <= NT:
                    t = k - 1
                    if t == LAST:
                        z2_tt(t, [(0, 352), (352, 1120), (1120, L_DVE)])
                        pa_map = [
                            (0, 1024, wp.tile([128, 1024], FP16,
                                              name="paL0", bufs=1)),
                            (1024, 1792, wp.tile([128, 768], FP16,
                                                 name="paL1", bufs=1)),
                            (1792, 2048, wp.tile([128, 256], FP16,
                                                 name="paL2", bufs=1)),
                        ]
                    else:
                        z2_tt(t, [(0, L_DVE)])
                        pa_map = [(0, 2048, wp.tile([128, 2048], FP16,
                                                    name=f"pa{t}", tag="pa",
                                                    bufs=4))]
                    exp_act(t, pa_map)
                if 2 <= k <= NT + 1:
                    exp_dve(k - 2)
                if k < NT:
                    pp = fills(k)
                    prelu_z1(k, pp)
                if k >= 3:
                    accum(k - 3)

            # ---- epilogue per accumulator bank: H = max(min(e^y-1, 0), y),
            # y = H_pre * (1/s) ----
            rc_all = op_pool.tile([128, NT], F32, name="rc_all")
            y_all = op_pool.tile([128, 1024], FP16, name="y_all")
            W_all = op_pool.tile([128, 1024], FP16, name="W_all")
            t1_all = op_pool.tile([128, 1024], FP16, name="t1_all")
            hout = op_pool.tile([128, 1024], FP16, name="hout")
            banks = [(0, 7), (7, 7), (14, 2)]  # (first tile, count)
            for bi, (m0, cnt) in enumerate(banks):
                a0 = 512 * bi
                g = cnt
                nc.vector.reciprocal(
                    rc_all[:, m0:m0 + g],
                    acc[:, a0 + 64:a0 + 64 + (g - 1) * 65 + 1:65])
                hp = acc[:, a0:a0 + g * 65] \
                    .rearrange("p (g c) -> p g c", c=65)[:, :, 0:64]
                rcb = rc_all[:, m0:m0 + g].unsqueeze(2) \
                    .broadcast_to([128, g, 64])
                y0 = 64 * m0
                ysl = y_all[:, y0:y0 + g * 64]
                nc.vector.tensor_tensor(
                    ysl.rearrange("p (g c) -> p g c", c=64), hp, rcb, ALU.mult)
                nc.scalar.activation(
                    W_all[:, y0:y0 + g * 64], ysl, AF.Exp, bias=0.0, scale=1.0)
                nc.vector.tensor_scalar(
                    t1_all[:, y0:y0 + g * 64], W_all[:, y0:y0 + g * 64],
                    -1.0, 0.0, ALU.add, ALU.min)
                nc.vector.tensor_tensor(
                    hout[:, y0:y0 + g * 64], t1_all[:, y0:y0 + g * 64],
                    y_all[:, y0:y0 + g * 64], ALU.max)
                nc.sync.dma_start(
                    H_d.ap()[128 * m0:128 * (m0 + g), :]
                    .rearrange("(t p) d -> p t d", p=128),
                    hout[:, y0:y0 + g * 64]
                    .rearrange("p (t d) -> p t d", d=D),
                )

    nc.compile()
    return nc


def _get_program():
    if "nc" not in _CACHED:
        _CACHED["nc"] = _build_program()
    return _CACHED["nc"]


def _host_prep(A, X, Ws, a):
    """Per-core host-side input preparation (cheap: ~67 MFLOP total)."""
    f64 = np.float64
    in_maps = []
    ci = (C_MASK * np.eye(128)).astype(ml_dtypes.bfloat16)
    for b in range(B):
        Wh = X[b].astype(f64) @ Ws.astype(f64)            # [N, D]
        Wh1 = (Wh @ a[:D].astype(f64))[:, 0]              # [N]
        Wh2 = (Wh @ a[D:].astype(f64))[:, 0]              # [N]
        S = max(0.0, float(Wh1.max() + Wh2.max()) - 10.5)
        whaug = np.ones((N, D + 1), np.float16)
        whaug[:, :D] = Wh.astype(np.float16)
        wh1_hi = Wh1.astype(ml_dtypes.bfloat16)
        wh1_lo = (Wh1 - wh1_hi.astype(f64)).astype(ml_dtypes.bfloat16)
        biasT = (Wh2 - C_MASK).astype(np.float32).reshape(NT, 128).T

        blobS = np.zeros((128, BS_W), np.float32)
        blobS[:, BS_BIAS:BS_BIAS + NT] = biasT
        blobS[:, BS_NEGS] = -S
        blobS.view(np.uint16)[:, 2 * BS_CI:2 * BS_CI + 128] = ci.view(np.uint16)

        blobW = np.zeros((128, 520), np.float32)
        blobW.view(np.uint16)[:, 0:1040] = whaug.reshape(NT, 128, D + 1) \
            .transpose(1, 0, 2).reshape(128, 1040).view(np.uint16)

        blob2 = np.zeros((2, 2176), ml_dtypes.bfloat16)
        blob2[0, 0:2048] = wh1_hi
        blob2[1, 0:2048] = wh1_lo
        blob2[:, 2048:2176] = np.ones((2, 128), ml_dtypes.bfloat16)

        in_maps.append({
            "A": np.ascontiguousarray(A[b]),
            "blobS": blobS,
            "blobW": blobW,
            "blob2": blob2,
        })
    return in_maps


def kernel(A, X, Ws, a, _trace=False, _trace_kwargs=None):
    A = np.asarray(A, np.float32)
    X = np.asarray(X, np.float32)
    Ws = np.asarray(Ws, np.float32)
    a = np.asarray(a, np.float32)
    nc = _get_program()
    in_maps = _host_prep(A, X, Ws, a)
    kw = {}
    if _trace:
        kw = {"trace": True, **(_trace_kwargs or {})}
    res = bass_utils.run_bass_kernel_spmd(nc, in_maps, core_ids=list(range(B)), **kw)
    H = np.stack([np.asarray(res.results[b]["H"], np.float32) for b in range(B)])
    if _trace:
        kernel.last_results = res
    return H
